# revision 33
# baseline (speedup 1.0000x reference)
"""Single-head causal attention (B=4, T=2048, D=1024, HS=64) on 8 TRN2 cores.

Sharding: 2 cores per batch element. Query blocks (128 rows, 16/batch) are
fold-split for perfect causal balance:
  role 0 (cores 0-3): blocks {0,1,2,3,12,13,14,15} of batch (core_id % 4)
  role 1 (cores 4-7): blocks {4..11}              of batch (core_id % 4)
The SPMD program branches on partition_id for the role-specific part.

Precision scheme (bf16 hi/lo pairs, ~17-18 effective mantissa bits):
  host: x.T split into bf16 hi/lo, interleaved per 512-column chunk
  (xhl[d, ch, 0:512]=hi, [512:1024]=lo) so DMA rows stay 2KB contiguous.
  W pre-transposed bf16 pairs.  Projections per 512-col chunk (role-dep):
    q-chunks   : lhsT=[Wq|Wk] 3 groups -> PSUM rows 0:64 q^T, 64:128 k^T,
                 plus a 1-group v-only pass (lhsT=[Wv_hi], v to bf16 acc).
    other chunk: lhsT=[Wv|Wk] 3 groups -> v^T, k^T.
  (3 groups = xh*wh + xl*wh + xh*wl accumulated in fp32 PSUM.)
  role 1 needs keys only to t<1536: its chunk-3 k/v pass is skipped.
  scores: k,q re-split into bf16 pairs on device;
    S = [qh;qh]^T.[kl;kh] + [0;ql]^T.[kl;kh]  (2 matmuls per 512-chunk)
  softmax: chunked row-max (DVE) + exp on ACT (scale=8, bias=-8*max),
  E bf16; E^T via PE transposes batched 4-per-PSUM-tile; out^T = v^T @ E^T
  in PSUM; 1/Z applied after a small back-transpose. Output fp32.
"""

import numpy as np

N_CORES = 8
B, T, D, HS = 4, 2048, 1024, 64
P = 128
NT = T // P        # 16
ND = D // P        # 8
NCH = 4            # 512-wide t chunks
SCALE = 8.0        # sqrt(HS)
NEG = -1.0e30

ROLE_BLOCKS = [
    [0, 12, 1, 13, 2, 14, 15, 3],
    [4, 8, 5, 9, 6, 10, 11, 7],
]
ROLE_QCHUNKS = [[0, 3], [1, 2]]  # 512-chunk indices holding each role's q rows
ROLE_NKV = [4, 3]                # k/v chunks needed per role


def _block_qloc(role, j):
    if role == 0:
        return (0, 128 * j) if j < 4 else (1, 128 * (j - 12))
    return (0, 128 * (j - 4)) if j < 8 else (1, 128 * (j - 8))


_COMPILED = None


def _build():
    import concourse.bass as bass
    import concourse.tile as tile
    from concourse import bacc, mybir

    f32 = mybir.dt.float32
    bf16 = mybir.dt.bfloat16
    EXP = mybir.ActivationFunctionType.Exp
    AX = mybir.AxisListType.X

    nc = bacc.Bacc("TRN2", target_bir_lowering=False, debug=False,
                   num_devices=N_CORES)

    xhl_d = nc.dram_tensor("xhl", [D, NCH, 1024], bf16,
                           kind="ExternalInput").ap()
    wvkh_d = nc.dram_tensor("wvkh", [P, ND * P], bf16, kind="ExternalInput").ap()
    wvkl_d = nc.dram_tensor("wvkl", [P, ND * P], bf16, kind="ExternalInput").ap()
    wqkh_d = nc.dram_tensor("wqkh", [P, ND * P], bf16, kind="ExternalInput").ap()
    wqkl_d = nc.dram_tensor("wqkl", [P, ND * P], bf16, kind="ExternalInput").ap()
    wvh_d = nc.dram_tensor("wvh", [P, ND * HS], bf16, kind="ExternalInput").ap()
    identb_d = nc.dram_tensor("identb", [P, P], bf16, kind="ExternalInput").ap()
    identf_d = nc.dram_tensor("identf", [HS, HS], f32, kind="ExternalInput").ap()
    mask_d = nc.dram_tensor("mask", [P, P], f32, kind="ExternalInput").ap()
    out_d = nc.dram_tensor("out", [1024, HS], f32, kind="ExternalOutput").ap()

    with tile.TileContext(nc) as tc:
        with tc.tile_pool(name="consts", bufs=1) as consts, \
             tc.tile_pool(name="big", bufs=1) as big:
            identb = consts.tile([P, P], bf16)
            identf = consts.tile([HS, HS], f32)
            mask = consts.tile([P, P], f32)
            wvkh = consts.tile([P, ND, P], bf16)
            wvkl = consts.tile([P, ND, P], bf16)
            wqkh = consts.tile([P, ND, P], bf16)
            wqkl = consts.tile([P, ND, P], bf16)
            wvh = consts.tile([P, ND, HS], bf16)

            # per-chunk x^T tiles: [:, dt, 0:512] = hi, [512:1024] = lo
            xc = [big.tile([P, ND, 1024], bf16, name=f"xc{ch}", tag=f"xc{ch}")
                  for ch in range(NCH)]
            # identity first (gates the PE warm-up chain), then x chunk 0
            nc.sync.dma_start(identb[:], identb_d[:])
            for dt in range(ND):
                nc.sync.dma_start(
                    xc[0][:, dt, :], xhl_d[dt * P:(dt + 1) * P, 0, :])
            nc.scalar.dma_start(wvkh[:], wvkh_d.rearrange("p (a h) -> p a h", a=ND))
            nc.scalar.dma_start(wvkl[:], wvkl_d.rearrange("p (a h) -> p a h", a=ND))
            nc.scalar.dma_start(wqkh[:], wqkh_d.rearrange("p (a h) -> p a h", a=ND))
            nc.scalar.dma_start(wqkl[:], wqkl_d.rearrange("p (a h) -> p a h", a=ND))
            nc.scalar.dma_start(wvh[:], wvh_d.rearrange("p (a h) -> p a h", a=ND))
            nc.scalar.dma_start(identf[:], identf_d[:])
            nc.scalar.dma_start(mask[:], mask_d[:])
            # KHL: rows 0:64 = k_lo, rows 64:128 = k_hi
            KHL = big.tile([P, T], bf16)
            vTb = big.tile([HS, T], bf16)
            vn = big.tile([P, NT, HS], bf16)
            qhh = big.tile([P, 2, 512], bf16)   # rows 0:64=q_hi, 64:128=q_hi
            qlz = big.tile([P, 2, 512], bf16)   # rows 0:64=0,    64:128=q_lo
            nc.vector.memset(qlz[0:HS, :, :], 0.0)

            for ch in range(1, NCH):
                for dt in range(ND):
                    nc.sync.dma_start(
                        xc[ch][:, dt, :], xhl_d[dt * P:(dt + 1) * P, ch, :])

            with tc.tile_pool(name="warm", bufs=1, space="PSUM") as warm:
                dps = warm.tile([P, P], f32, name="dps")
                for _ in range(72):
                    nc.tensor.matmul(dps[:], lhsT=identb[:], rhs=identb[:],
                                     start=True, stop=True)

            with tc.tile_pool(name="spool", bufs=5, space="PSUM") as spool, \
                 tc.tile_pool(name="etp", bufs=2, space="PSUM") as etp, \
                 tc.tile_pool(name="avp", bufs=1, space="PSUM") as avp, \
                 tc.tile_pool(name="epool", bufs=4) as epool, \
                 tc.tile_pool(name="ets", bufs=6) as ets, \
                 tc.tile_pool(name="small", bufs=4) as small, \
                 tc.tile_pool(name="osb", bufs=3) as osb, \
                 tc.tile_pool(name="qtmp", bufs=3) as qtp:

                def emit_proj(role):
                    qchunks = ROLE_QCHUNKS[role]
                    for ch in range(ROLE_NKV[role]):
                        cs = slice(ch * 512, (ch + 1) * 512)
                        is_q = ch in qchunks
                        w_h, w_l = (wqkh, wqkl) if is_q else (wvkh, wvkl)
                        ps = spool.tile([P, 512], f32, tag="S")
                        ngrp = [(w_h, 0), (w_h, 512), (w_l, 0)]
                        n = len(ngrp) * ND
                        i = 0
                        for w_t, xoff in ngrp:
                            for dt in range(ND):
                                nc.tensor.matmul(
                                    ps[:], lhsT=w_t[:, dt, :],
                                    rhs=xc[ch][:, dt, xoff:xoff + 512],
                                    start=(i == 0), stop=(i == n - 1))
                                i += 1
                        # rows 64:128 = k^T always; split k into bf16 pair
                        nc.scalar.copy(KHL[HS:P, cs], ps[HS:P, :])
                        kl = qtp.tile([P, 512], bf16, tag="kl")
                        nc.vector.tensor_sub(kl[HS:P, :], ps[HS:P, :],
                                             KHL[HS:P, cs])
                        nc.gpsimd.dma_start(KHL[0:HS, cs], kl[HS:P, :])
                        if is_q:
                            qc = qchunks.index(ch)
                            nc.scalar.copy(qhh[0:HS, qc, :], ps[0:HS, :])
                            qt = qtp.tile([HS, 512], bf16, tag="qt")
                            nc.vector.tensor_sub(qt[:], ps[0:HS, :],
                                                 qhh[0:HS, qc, :])
                            nc.gpsimd.dma_start(qhh[HS:P, qc, :],
                                                qhh[0:HS, qc, :])
                            nc.gpsimd.dma_start(qlz[HS:P, qc, :], qt[:])
                            # v-only single-group pass for this chunk
                            psv = spool.tile([HS, 512], f32, tag="S")
                            for dt in range(ND):
                                nc.tensor.matmul(
                                    psv[:], lhsT=wvh[:, dt, :],
                                    rhs=xc[ch][:, dt, 0:512],
                                    start=(dt == 0), stop=(dt == ND - 1))
                            nc.vector.tensor_copy(vTb[:, cs], psv[:])
                        else:
                            nc.vector.tensor_copy(vTb[:, cs], ps[0:HS, :])
                        # v^T -> v natural (bf16), 4 transposes in one tile
                        vp = etp.tile([P, 512], bf16, tag="ep")
                        for u in range(4):
                            tt = ch * 4 + u
                            nc.tensor.transpose(
                                vp[:, u * HS:u * HS + HS],
                                vTb[:, tt * P:(tt + 1) * P],
                                identb[0:HS, 0:HS])
                        nc.scalar.copy(
                            vn[:, ch * 4:(ch + 1) * 4, :],
                            vp[:, 0:4 * HS].rearrange("p (a h) -> p a h", a=4))

                def emit_role(role):
                    emit_proj(role)

                    def chunk_scores(st, kc):
                        L = st["L"]
                        w = min(512, L - kc * 512)
                        sp = spool.tile([P, w], f32, tag="S")
                        rhs = KHL[:, kc * 512:kc * 512 + w]
                        nc.tensor.matmul(sp[:], lhsT=st["qh"], rhs=rhs,
                                         start=True, stop=False)
                        nc.tensor.matmul(sp[:], lhsT=st["ql"], rhs=rhs,
                                         start=False, stop=True)
                        if kc == st["nch"] - 1:
                            nc.vector.tensor_add(
                                sp[:, w - P:w], sp[:, w - P:w], mask[:])
                        nc.vector.reduce_max(
                            st["mc"][:, kc:kc + 1], sp[:], axis=AX)
                        st["sps"].append((sp, w))

                    def begin_block(slot, j):
                        L = 128 * (j + 1)
                        qc, off = _block_qloc(role, j)
                        st = {"slot": slot, "j": j, "L": L,
                              "qh": qhh[:, qc, off:off + 128],
                              "ql": qlz[:, qc, off:off + 128],
                              "nch": (L + 511) // 512, "sps": []}
                        st["mc"] = small.tile([P, st["nch"]], f32, tag="mc",
                                              name="mc")
                        chunk_scores(st, 0)
                        return st

                    def emit_tail(st):
                        slot, L, nch = st["slot"], st["L"], st["nch"]
                        nm8 = small.tile([P, 1], f32, tag="nm8")
                        if nch == 1:
                            nc.vector.tensor_scalar_mul(
                                nm8[:], st["mc"][:, 0:1], -SCALE)
                        else:
                            m = small.tile([P, 1], f32, tag="m")
                            nc.vector.reduce_max(m[:], st["mc"][:], axis=AX)
                            nc.vector.tensor_scalar_mul(nm8[:], m[:], -SCALE)

                        E = epool.tile([P, L], bf16, tag="E")
                        zc = small.tile([P, nch], f32, tag="zc", name="zc")
                        for kc, (sp, w) in enumerate(st["sps"]):
                            nc.scalar.activation(
                                E[:, kc * 512:kc * 512 + w], sp[:], EXP,
                                bias=nm8[:], scale=SCALE,
                                accum_out=zc[:, kc:kc + 1])

                        rz = small.tile([P, 1], f32, tag="rz")
                        if nch == 1:
                            nc.vector.reciprocal(rz[:], zc[:, 0:1])
                        else:
                            zs = small.tile([P, 1], f32, tag="zs")
                            nc.vector.reduce_sum(zs[:], zc[:], axis=AX)
                            nc.vector.reciprocal(rz[:], zs[:])

                        # E^T via PE transposes, 8 per PSUM tile, one copy
                        av = avp.tile([HS, P], f32, tag="av")
                        nkt = L // P
                        kt = 0
                        gi = 0
                        while kt < nkt:
                            gn = min(8, nkt - kt)
                            ep = etp.tile([P, 1024], bf16, tag="ep")
                            for u in range(gn):
                                nc.tensor.transpose(
                                    ep[:, u * P:(u + 1) * P],
                                    E[:, (kt + u) * P:(kt + u + 1) * P],
                                    identb[:])
                            es = ets.tile([P, 1024], bf16, tag="ets")
                            if gi % 3 == 0:
                                nc.scalar.copy(es[:, 0:gn * P],
                                               ep[:, 0:gn * P])
                            else:
                                nc.vector.tensor_copy(es[:, 0:gn * P],
                                                      ep[:, 0:gn * P])
                            for u in range(gn):
                                nc.tensor.matmul(
                                    av[:], lhsT=vn[:, kt + u, :],
                                    rhs=es[:, u * P:(u + 1) * P],
                                    start=(kt + u == 0),
                                    stop=(kt + u == nkt - 1),
                                    skip_group_check=True)
                            kt += gn
                            gi += 1

                        avs = osb.tile([HS, P], f32, tag="avs")
                        nc.vector.tensor_copy(avs[:], av[:])
                        op = etp.tile([P, HS], f32, tag="ep")
                        nc.tensor.transpose(op[:], avs[:], identf[:])
                        ob = osb.tile([P, HS], f32, tag="ob")
                        nc.vector.tensor_scalar_mul(ob[:], op[:], rz[:])
                        nc.sync.dma_start(
                            out_d[slot * P:(slot + 1) * P, :], ob[:])

                    prev = None
                    for slot, j in enumerate(ROLE_BLOCKS[role]):
                        st = begin_block(slot, j)
                        if prev is not None:
                            emit_tail(prev)
                        for kc in range(1, st["nch"]):
                            chunk_scores(st, kc)
                        prev = st
                    emit_tail(prev)

                pid = nc.partition_id()
                with tc.If(pid < 4) as cmp:
                    emit_role(0)
                with cmp.Else():
                    emit_role(1)

    nc.compile()
    return nc


def _get_program():
    global _COMPILED
    if _COMPILED is None:
        _COMPILED = _build()
    return _COMPILED


def _install_ntff_hook():
    import sys, types
    if "antenv.axon_hooks" in sys.modules:
        return
    try:
        from trn_agent_boot.trn_boot import _ntff_profile_via_ctypes
        hook = _ntff_profile_via_ctypes("/opt/axon/libaxon_pjrt.so")
        mod = types.ModuleType("antenv.axon_hooks")
        mod.get_axon_ntff_profile_hook = lambda: hook
        mod.set_axon_ntff_profile_hook = lambda h: None
        import antenv
        sys.modules["antenv.axon_hooks"] = mod
        antenv.axon_hooks = mod
    except Exception:
        pass


def _split_pair(a):
    import ml_dtypes
    hi = a.astype(ml_dtypes.bfloat16)
    lo = (a - hi.astype(np.float32)).astype(ml_dtypes.bfloat16)
    return hi, lo


def _host_prep(inputs):
    import ml_dtypes
    x = np.asarray(inputs["x"], dtype=np.float32)
    wq = np.asarray(inputs["Wq"], dtype=np.float32)
    wk = np.asarray(inputs["Wk"], dtype=np.float32)
    wv = np.asarray(inputs["Wv"], dtype=np.float32)

    xt = np.transpose(x, (0, 2, 1))                # [B, D, T]
    xtc = xt.reshape(B, D, NCH, 512)               # chunked over T
    hi, lo = _split_pair(np.ascontiguousarray(xtc))
    xhl = np.concatenate([hi, lo], axis=3)         # [B, D, NCH, 1024]

    def _wprep(wt):
        # [D, M] -> [P, ND*M]: row p holds [dt, m] for d = dt*P + p
        m = wt.shape[1]
        return np.ascontiguousarray(
            wt.reshape(ND, P, m).transpose(1, 0, 2).reshape(P, ND * m))

    wvkh, wvkl = _split_pair(_wprep(np.concatenate([wv, wk], axis=0).T))
    wqkh, wqkl = _split_pair(_wprep(np.concatenate([wq, wk], axis=0).T))
    wvhh, _ = _split_pair(_wprep(wv.T))

    identb = np.eye(P, dtype=ml_dtypes.bfloat16)
    identf = np.eye(HS, dtype=np.float32)
    r = np.arange(P)
    mask = np.where(r[None, :] <= r[:, None], 0.0, NEG).astype(np.float32)

    shared = {"wvkh": wvkh, "wvkl": wvkl, "wqkh": wqkh, "wqkl": wqkl,
              "wvh": wvhh, "identb": identb, "identf": identf, "mask": mask}
    in_maps = []
    for c in range(N_CORES):
        m = dict(shared)
        m["xhl"] = np.ascontiguousarray(xhl[c % B])
        in_maps.append(m)
    return in_maps


def _run(inputs, trace=False):
    from concourse.bass_utils import run_bass_kernel_spmd

    if trace:
        _install_ntff_hook()
    nc = _get_program()
    in_maps = _host_prep(inputs)
    res = run_bass_kernel_spmd(nc, in_maps, list(range(N_CORES)), trace=trace)

    out = np.empty((B, T, HS), dtype=np.float32)
    for c in range(N_CORES):
        b, role = c % B, c // B
        oc = res.results[c]["out"]
        for slot, j in enumerate(ROLE_BLOCKS[role]):
            out[b, 128 * j:128 * (j + 1)] = oc[128 * slot:128 * (slot + 1)]
    return out, res


def kernel(**inputs):
    out, _ = _run(inputs, trace=False)
    return out


# revision 34
# speedup vs baseline: 1.0289x; 1.0289x over previous
"""Single-head causal attention (B=4, T=2048, D=1024, HS=64) on 8 TRN2 cores.

Sharding: 2 cores per batch element. Query blocks (128 rows, 16/batch) are
fold-split for perfect causal balance:
  role 0 (cores 0-3): blocks {0,1,2,3,12,13,14,15} of batch (core_id % 4)
  role 1 (cores 4-7): blocks {4..11}              of batch (core_id % 4)
The SPMD program branches on partition_id for the role-specific part.

Precision scheme (bf16 hi/lo pairs, ~17-18 effective mantissa bits):
  host: x.T split into bf16 hi/lo, interleaved per 512-column chunk
  (xhl[d, ch, 0:512]=hi, [512:1024]=lo) so DMA rows stay 2KB contiguous.
  W pre-transposed bf16 pairs.  Projections per 512-col chunk (role-dep):
    q-chunks   : lhsT=[Wq|Wk] 3 groups -> PSUM rows 0:64 q^T, 64:128 k^T,
                 plus a 1-group v-only pass (lhsT=[Wv_hi], v to bf16 acc).
    other chunk: lhsT=[Wv|Wk] 3 groups -> v^T, k^T.
  (3 groups = xh*wh + xl*wh + xh*wl accumulated in fp32 PSUM.)
  role 1 needs keys only to t<1536: its chunk-3 k/v pass is skipped.
  scores: k,q re-split into bf16 pairs on device;
    S = [qh;qh]^T.[kl;kh] + [0;ql]^T.[kl;kh]  (2 matmuls per 512-chunk)
  softmax: chunked row-max (DVE) + exp on ACT (scale=8, bias=-8*max),
  E bf16; E^T via PE transposes batched 4-per-PSUM-tile; out^T = v^T @ E^T
  in PSUM; 1/Z applied after a small back-transpose. Output fp32.
"""

import numpy as np

N_CORES = 8
B, T, D, HS = 4, 2048, 1024, 64
P = 128
NT = T // P        # 16
ND = D // P        # 8
NCH = 4            # 512-wide t chunks
SCALE = 8.0        # sqrt(HS)
NEG = -1.0e30

ROLE_BLOCKS = [
    [3, 12, 1, 13, 2, 14, 15, 0],
    [4, 8, 5, 9, 6, 10, 11, 7],
]
ROLE_QCHUNKS = [[0, 3], [1, 2]]  # 512-chunk indices holding each role's q rows
ROLE_NKV = [4, 3]                # k/v chunks needed per role


def _block_qloc(role, j):
    if role == 0:
        return (0, 128 * j) if j < 4 else (1, 128 * (j - 12))
    return (0, 128 * (j - 4)) if j < 8 else (1, 128 * (j - 8))


_COMPILED = None


def _build():
    import concourse.bass as bass
    import concourse.tile as tile
    from concourse import bacc, mybir

    f32 = mybir.dt.float32
    bf16 = mybir.dt.bfloat16
    EXP = mybir.ActivationFunctionType.Exp
    AX = mybir.AxisListType.X

    nc = bacc.Bacc("TRN2", target_bir_lowering=False, debug=False,
                   num_devices=N_CORES)

    xhl_d = nc.dram_tensor("xhl", [D, NCH, 1024], bf16,
                           kind="ExternalInput").ap()
    wvkh_d = nc.dram_tensor("wvkh", [P, ND * P], bf16, kind="ExternalInput").ap()
    wvkl_d = nc.dram_tensor("wvkl", [P, ND * P], bf16, kind="ExternalInput").ap()
    wqkh_d = nc.dram_tensor("wqkh", [P, ND * P], bf16, kind="ExternalInput").ap()
    wqkl_d = nc.dram_tensor("wqkl", [P, ND * P], bf16, kind="ExternalInput").ap()
    wvh_d = nc.dram_tensor("wvh", [P, ND * HS], bf16, kind="ExternalInput").ap()
    identb_d = nc.dram_tensor("identb", [P, P], bf16, kind="ExternalInput").ap()
    identf_d = nc.dram_tensor("identf", [HS, HS], f32, kind="ExternalInput").ap()
    mask_d = nc.dram_tensor("mask", [P, P], f32, kind="ExternalInput").ap()
    out_d = nc.dram_tensor("out", [1024, HS], f32, kind="ExternalOutput").ap()

    with tile.TileContext(nc) as tc:
        with tc.tile_pool(name="consts", bufs=1) as consts, \
             tc.tile_pool(name="big", bufs=1) as big:
            identb = consts.tile([P, P], bf16)
            identf = consts.tile([HS, HS], f32)
            mask = consts.tile([P, P], f32)
            wvkh = consts.tile([P, ND, P], bf16)
            wvkl = consts.tile([P, ND, P], bf16)
            wqkh = consts.tile([P, ND, P], bf16)
            wqkl = consts.tile([P, ND, P], bf16)
            wvh = consts.tile([P, ND, HS], bf16)

            # per-chunk x^T tiles: [:, dt, 0:512] = hi, [512:1024] = lo
            xc = [big.tile([P, ND, 1024], bf16, name=f"xc{ch}", tag=f"xc{ch}")
                  for ch in range(NCH)]
            # identity first (gates the PE warm-up chain), then x chunk 0
            nc.sync.dma_start(identb[:], identb_d[:])
            for dt in range(ND):
                nc.sync.dma_start(
                    xc[0][:, dt, :], xhl_d[dt * P:(dt + 1) * P, 0, :])
            nc.scalar.dma_start(wvkh[:], wvkh_d.rearrange("p (a h) -> p a h", a=ND))
            nc.scalar.dma_start(wvkl[:], wvkl_d.rearrange("p (a h) -> p a h", a=ND))
            nc.scalar.dma_start(wqkh[:], wqkh_d.rearrange("p (a h) -> p a h", a=ND))
            nc.scalar.dma_start(wqkl[:], wqkl_d.rearrange("p (a h) -> p a h", a=ND))
            nc.scalar.dma_start(wvh[:], wvh_d.rearrange("p (a h) -> p a h", a=ND))
            nc.scalar.dma_start(identf[:], identf_d[:])
            nc.scalar.dma_start(mask[:], mask_d[:])
            # KHL: rows 0:64 = k_lo, rows 64:128 = k_hi
            KHL = big.tile([P, T], bf16)
            vTb = big.tile([HS, T], bf16)
            vn = big.tile([P, NT, HS], bf16)
            qhh = big.tile([P, 2, 512], bf16)   # rows 0:64=q_hi, 64:128=q_hi
            qlz = big.tile([P, 2, 512], bf16)   # rows 0:64=0,    64:128=q_lo
            nc.vector.memset(qlz[0:HS, :, :], 0.0)

            for ch in range(1, NCH):
                for dt in range(ND):
                    nc.sync.dma_start(
                        xc[ch][:, dt, :], xhl_d[dt * P:(dt + 1) * P, ch, :])

            with tc.tile_pool(name="warm", bufs=1, space="PSUM") as warm:
                dps = warm.tile([P, P], f32, name="dps")
                for _ in range(72):
                    nc.tensor.matmul(dps[:], lhsT=identb[:], rhs=identb[:],
                                     start=True, stop=True)

            with tc.tile_pool(name="spool", bufs=5, space="PSUM") as spool, \
                 tc.tile_pool(name="etp", bufs=2, space="PSUM") as etp, \
                 tc.tile_pool(name="avp", bufs=1, space="PSUM") as avp, \
                 tc.tile_pool(name="epool", bufs=4) as epool, \
                 tc.tile_pool(name="ets", bufs=6) as ets, \
                 tc.tile_pool(name="small", bufs=4) as small, \
                 tc.tile_pool(name="osb", bufs=3) as osb, \
                 tc.tile_pool(name="qtmp", bufs=3) as qtp:

                def emit_proj(role):
                    qchunks = ROLE_QCHUNKS[role]
                    for ch in range(ROLE_NKV[role]):
                        cs = slice(ch * 512, (ch + 1) * 512)
                        is_q = ch in qchunks
                        w_h, w_l = (wqkh, wqkl) if is_q else (wvkh, wvkl)
                        ps = spool.tile([P, 512], f32, tag="S")
                        ngrp = [(w_h, 0), (w_h, 512), (w_l, 0)]
                        n = len(ngrp) * ND
                        i = 0
                        for w_t, xoff in ngrp:
                            for dt in range(ND):
                                nc.tensor.matmul(
                                    ps[:], lhsT=w_t[:, dt, :],
                                    rhs=xc[ch][:, dt, xoff:xoff + 512],
                                    start=(i == 0), stop=(i == n - 1))
                                i += 1
                        # rows 64:128 = k^T always; split k into bf16 pair
                        nc.scalar.copy(KHL[HS:P, cs], ps[HS:P, :])
                        kl = qtp.tile([P, 512], bf16, tag="kl")
                        nc.vector.tensor_sub(kl[HS:P, :], ps[HS:P, :],
                                             KHL[HS:P, cs])
                        nc.gpsimd.dma_start(KHL[0:HS, cs], kl[HS:P, :])
                        if is_q:
                            qc = qchunks.index(ch)
                            nc.scalar.copy(qhh[0:HS, qc, :], ps[0:HS, :])
                            qt = qtp.tile([HS, 512], bf16, tag="qt")
                            nc.vector.tensor_sub(qt[:], ps[0:HS, :],
                                                 qhh[0:HS, qc, :])
                            nc.gpsimd.dma_start(qhh[HS:P, qc, :],
                                                qhh[0:HS, qc, :])
                            nc.gpsimd.dma_start(qlz[HS:P, qc, :], qt[:])
                            # v-only single-group pass for this chunk
                            psv = spool.tile([HS, 512], f32, tag="S")
                            for dt in range(ND):
                                nc.tensor.matmul(
                                    psv[:], lhsT=wvh[:, dt, :],
                                    rhs=xc[ch][:, dt, 0:512],
                                    start=(dt == 0), stop=(dt == ND - 1))
                            nc.vector.tensor_copy(vTb[:, cs], psv[:])
                        else:
                            nc.vector.tensor_copy(vTb[:, cs], ps[0:HS, :])
                        # v^T -> v natural (bf16), 4 transposes in one tile
                        vp = etp.tile([P, 512], bf16, tag="ep")
                        for u in range(4):
                            tt = ch * 4 + u
                            nc.tensor.transpose(
                                vp[:, u * HS:u * HS + HS],
                                vTb[:, tt * P:(tt + 1) * P],
                                identb[0:HS, 0:HS])
                        nc.scalar.copy(
                            vn[:, ch * 4:(ch + 1) * 4, :],
                            vp[:, 0:4 * HS].rearrange("p (a h) -> p a h", a=4))

                def emit_role(role):
                    emit_proj(role)

                    def chunk_scores(st, kc):
                        L = st["L"]
                        w = min(512, L - kc * 512)
                        sp = spool.tile([P, w], f32, tag="S")
                        rhs = KHL[:, kc * 512:kc * 512 + w]
                        nc.tensor.matmul(sp[:], lhsT=st["qh"], rhs=rhs,
                                         start=True, stop=False)
                        nc.tensor.matmul(sp[:], lhsT=st["ql"], rhs=rhs,
                                         start=False, stop=True)
                        if kc == st["nch"] - 1:
                            nc.vector.tensor_add(
                                sp[:, w - P:w], sp[:, w - P:w], mask[:])
                        nc.vector.reduce_max(
                            st["mc"][:, kc:kc + 1], sp[:], axis=AX)
                        st["sps"].append((sp, w))

                    def begin_block(slot, j):
                        L = 128 * (j + 1)
                        qc, off = _block_qloc(role, j)
                        st = {"slot": slot, "j": j, "L": L,
                              "qh": qhh[:, qc, off:off + 128],
                              "ql": qlz[:, qc, off:off + 128],
                              "nch": (L + 511) // 512, "sps": []}
                        st["mc"] = small.tile([P, st["nch"]], f32, tag="mc",
                                              name="mc")
                        chunk_scores(st, 0)
                        return st

                    def emit_tail(st):
                        slot, L, nch = st["slot"], st["L"], st["nch"]
                        nm8 = small.tile([P, 1], f32, tag="nm8")
                        if nch == 1:
                            nc.vector.tensor_scalar_mul(
                                nm8[:], st["mc"][:, 0:1], -SCALE)
                        else:
                            m = small.tile([P, 1], f32, tag="m")
                            nc.vector.reduce_max(m[:], st["mc"][:], axis=AX)
                            nc.vector.tensor_scalar_mul(nm8[:], m[:], -SCALE)

                        E = epool.tile([P, L], bf16, tag="E")
                        zc = small.tile([P, nch], f32, tag="zc", name="zc")
                        for kc, (sp, w) in enumerate(st["sps"]):
                            nc.scalar.activation(
                                E[:, kc * 512:kc * 512 + w], sp[:], EXP,
                                bias=nm8[:], scale=SCALE,
                                accum_out=zc[:, kc:kc + 1])

                        rz = small.tile([P, 1], f32, tag="rz")
                        if nch == 1:
                            nc.vector.reciprocal(rz[:], zc[:, 0:1])
                        else:
                            zs = small.tile([P, 1], f32, tag="zs")
                            nc.vector.reduce_sum(zs[:], zc[:], axis=AX)
                            nc.vector.reciprocal(rz[:], zs[:])

                        # E^T via PE transposes, 8 per PSUM tile, one copy
                        av = avp.tile([HS, P], f32, tag="av")
                        nkt = L // P
                        kt = 0
                        gi = 0
                        while kt < nkt:
                            gn = min(8, nkt - kt)
                            ep = etp.tile([P, 1024], bf16, tag="ep")
                            for u in range(gn):
                                nc.tensor.transpose(
                                    ep[:, u * P:(u + 1) * P],
                                    E[:, (kt + u) * P:(kt + u + 1) * P],
                                    identb[:])
                            es = ets.tile([P, 1024], bf16, tag="ets")
                            if gi % 3 == 0:
                                nc.scalar.copy(es[:, 0:gn * P],
                                               ep[:, 0:gn * P])
                            else:
                                nc.vector.tensor_copy(es[:, 0:gn * P],
                                                      ep[:, 0:gn * P])
                            for u in range(gn):
                                nc.tensor.matmul(
                                    av[:], lhsT=vn[:, kt + u, :],
                                    rhs=es[:, u * P:(u + 1) * P],
                                    start=(kt + u == 0),
                                    stop=(kt + u == nkt - 1),
                                    skip_group_check=True)
                            kt += gn
                            gi += 1

                        avs = osb.tile([HS, P], f32, tag="avs")
                        nc.vector.tensor_copy(avs[:], av[:])
                        op = etp.tile([P, HS], f32, tag="ep")
                        nc.tensor.transpose(op[:], avs[:], identf[:])
                        ob = osb.tile([P, HS], f32, tag="ob")
                        nc.vector.tensor_scalar_mul(ob[:], op[:], rz[:])
                        nc.sync.dma_start(
                            out_d[slot * P:(slot + 1) * P, :], ob[:])

                    prev = None
                    for slot, j in enumerate(ROLE_BLOCKS[role]):
                        st = begin_block(slot, j)
                        if prev is not None:
                            emit_tail(prev)
                        for kc in range(1, st["nch"]):
                            chunk_scores(st, kc)
                        prev = st
                    emit_tail(prev)

                pid = nc.partition_id()
                with tc.If(pid < 4) as cmp:
                    emit_role(0)
                with cmp.Else():
                    emit_role(1)

    nc.compile()
    return nc


def _get_program():
    global _COMPILED
    if _COMPILED is None:
        _COMPILED = _build()
    return _COMPILED


def _install_ntff_hook():
    import sys, types
    if "antenv.axon_hooks" in sys.modules:
        return
    try:
        from trn_agent_boot.trn_boot import _ntff_profile_via_ctypes
        hook = _ntff_profile_via_ctypes("/opt/axon/libaxon_pjrt.so")
        mod = types.ModuleType("antenv.axon_hooks")
        mod.get_axon_ntff_profile_hook = lambda: hook
        mod.set_axon_ntff_profile_hook = lambda h: None
        import antenv
        sys.modules["antenv.axon_hooks"] = mod
        antenv.axon_hooks = mod
    except Exception:
        pass


def _split_pair(a):
    import ml_dtypes
    hi = a.astype(ml_dtypes.bfloat16)
    lo = (a - hi.astype(np.float32)).astype(ml_dtypes.bfloat16)
    return hi, lo


def _host_prep(inputs):
    import ml_dtypes
    x = np.asarray(inputs["x"], dtype=np.float32)
    wq = np.asarray(inputs["Wq"], dtype=np.float32)
    wk = np.asarray(inputs["Wk"], dtype=np.float32)
    wv = np.asarray(inputs["Wv"], dtype=np.float32)

    xt = np.transpose(x, (0, 2, 1))                # [B, D, T]
    xtc = xt.reshape(B, D, NCH, 512)               # chunked over T
    hi, lo = _split_pair(np.ascontiguousarray(xtc))
    xhl = np.concatenate([hi, lo], axis=3)         # [B, D, NCH, 1024]

    def _wprep(wt):
        # [D, M] -> [P, ND*M]: row p holds [dt, m] for d = dt*P + p
        m = wt.shape[1]
        return np.ascontiguousarray(
            wt.reshape(ND, P, m).transpose(1, 0, 2).reshape(P, ND * m))

    wvkh, wvkl = _split_pair(_wprep(np.concatenate([wv, wk], axis=0).T))
    wqkh, wqkl = _split_pair(_wprep(np.concatenate([wq, wk], axis=0).T))
    wvhh, _ = _split_pair(_wprep(wv.T))

    identb = np.eye(P, dtype=ml_dtypes.bfloat16)
    identf = np.eye(HS, dtype=np.float32)
    r = np.arange(P)
    mask = np.where(r[None, :] <= r[:, None], 0.0, NEG).astype(np.float32)

    shared = {"wvkh": wvkh, "wvkl": wvkl, "wqkh": wqkh, "wqkl": wqkl,
              "wvh": wvhh, "identb": identb, "identf": identf, "mask": mask}
    in_maps = []
    for c in range(N_CORES):
        m = dict(shared)
        m["xhl"] = np.ascontiguousarray(xhl[c % B])
        in_maps.append(m)
    return in_maps


def _run(inputs, trace=False):
    from concourse.bass_utils import run_bass_kernel_spmd

    if trace:
        _install_ntff_hook()
    nc = _get_program()
    in_maps = _host_prep(inputs)
    res = run_bass_kernel_spmd(nc, in_maps, list(range(N_CORES)), trace=trace)

    out = np.empty((B, T, HS), dtype=np.float32)
    for c in range(N_CORES):
        b, role = c % B, c // B
        oc = res.results[c]["out"]
        for slot, j in enumerate(ROLE_BLOCKS[role]):
            out[b, 128 * j:128 * (j + 1)] = oc[128 * slot:128 * (slot + 1)]
    return out, res


def kernel(**inputs):
    out, _ = _run(inputs, trace=False)
    return out


# revision 35
# speedup vs baseline: 1.0455x; 1.0162x over previous
"""Single-head causal attention (B=4, T=2048, D=1024, HS=64) on 8 TRN2 cores.

Sharding: 2 cores per batch element. Query blocks (128 rows, 16/batch) are
fold-split for perfect causal balance:
  role 0 (cores 0-3): blocks {0,1,2,3,12,13,14,15} of batch (core_id % 4)
  role 1 (cores 4-7): blocks {4..11}              of batch (core_id % 4)
The SPMD program branches on partition_id for the role-specific part.

Precision scheme (bf16 hi/lo pairs, ~17-18 effective mantissa bits):
  host: x.T split into bf16 hi/lo, interleaved per 512-column chunk
  (xhl[d, ch, 0:512]=hi, [512:1024]=lo) so DMA rows stay 2KB contiguous.
  W pre-transposed bf16 pairs.  Projections per 512-col chunk (role-dep):
    q-chunks   : lhsT=[Wq|Wk] 3 groups -> PSUM rows 0:64 q^T, 64:128 k^T,
                 plus a 1-group v-only pass (lhsT=[Wv_hi], v to bf16 acc).
    other chunk: lhsT=[Wv|Wk] 3 groups -> v^T, k^T.
  (3 groups = xh*wh + xl*wh + xh*wl accumulated in fp32 PSUM.)
  role 1 needs keys only to t<1536: its chunk-3 k/v pass is skipped.
  scores: k,q re-split into bf16 pairs on device;
    S = [qh;qh]^T.[kl;kh] + [0;ql]^T.[kl;kh]  (2 matmuls per 512-chunk)
  softmax: chunked row-max (DVE) + exp on ACT (scale=8, bias=-8*max),
  E bf16; E^T via PE transposes batched 4-per-PSUM-tile; out^T = v^T @ E^T
  in PSUM; 1/Z applied after a small back-transpose. Output fp32.
"""

import numpy as np

N_CORES = 8
B, T, D, HS = 4, 2048, 1024, 64
P = 128
NT = T // P        # 16
ND = D // P        # 8
NCH = 4            # 512-wide t chunks
SCALE = 8.0        # sqrt(HS)
NEG = -1.0e30

ROLE_BLOCKS = [
    [3, 12, 1, 13, 2, 14, 15, 0],
    [4, 8, 5, 9, 6, 10, 11, 7],
]
ROLE_QCHUNKS = [[0, 3], [1, 2]]  # 512-chunk indices holding each role's q rows
ROLE_NKV = [4, 3]                # k/v chunks needed per role


def _block_qloc(role, j):
    if role == 0:
        return (0, 128 * j) if j < 4 else (1, 128 * (j - 12))
    return (0, 128 * (j - 4)) if j < 8 else (1, 128 * (j - 8))


_COMPILED = None


def _build():
    import concourse.bass as bass
    import concourse.tile as tile
    from concourse import bacc, mybir

    f32 = mybir.dt.float32
    bf16 = mybir.dt.bfloat16
    EXP = mybir.ActivationFunctionType.Exp
    AX = mybir.AxisListType.X

    nc = bacc.Bacc("TRN2", target_bir_lowering=False, debug=False,
                   num_devices=N_CORES)

    xhl_d = nc.dram_tensor("xhl", [D, NCH, 1024], bf16,
                           kind="ExternalInput").ap()
    wvkh_d = nc.dram_tensor("wvkh", [P, ND * P], bf16, kind="ExternalInput").ap()
    wvkl_d = nc.dram_tensor("wvkl", [P, ND * P], bf16, kind="ExternalInput").ap()
    wqkh_d = nc.dram_tensor("wqkh", [P, ND * P], bf16, kind="ExternalInput").ap()
    wqkl_d = nc.dram_tensor("wqkl", [P, ND * P], bf16, kind="ExternalInput").ap()
    wvh_d = nc.dram_tensor("wvh", [P, ND * HS], bf16, kind="ExternalInput").ap()
    identb_d = nc.dram_tensor("identb", [P, P], bf16, kind="ExternalInput").ap()
    identf_d = nc.dram_tensor("identf", [HS, HS], f32, kind="ExternalInput").ap()
    mask_d = nc.dram_tensor("mask", [P, P], f32, kind="ExternalInput").ap()
    out_d = nc.dram_tensor("out", [1024, HS], f32, kind="ExternalOutput").ap()

    with tile.TileContext(nc) as tc:
        with tc.tile_pool(name="consts", bufs=1) as consts, \
             tc.tile_pool(name="big", bufs=1) as big:
            identb = consts.tile([P, P], bf16)
            identf = consts.tile([HS, HS], f32)
            mask = consts.tile([P, P], f32)
            wvkh = consts.tile([P, ND, P], bf16)
            wvkl = consts.tile([P, ND, P], bf16)
            wqkh = consts.tile([P, ND, P], bf16)
            wqkl = consts.tile([P, ND, P], bf16)
            wvh = consts.tile([P, ND, HS], bf16)

            # per-chunk x^T tiles: [:, dt, 0:512] = hi, [512:1024] = lo
            xc = [big.tile([P, ND, 1024], bf16, name=f"xc{ch}", tag=f"xc{ch}")
                  for ch in range(NCH)]
            # identity first (gates the PE warm-up chain), then x chunk 0
            nc.sync.dma_start(identb[:], identb_d[:])
            for dt in range(ND):
                nc.sync.dma_start(
                    xc[0][:, dt, :], xhl_d[dt * P:(dt + 1) * P, 0, :])
            nc.scalar.dma_start(wvkh[:], wvkh_d.rearrange("p (a h) -> p a h", a=ND))
            nc.scalar.dma_start(wvkl[:], wvkl_d.rearrange("p (a h) -> p a h", a=ND))
            nc.scalar.dma_start(wqkh[:], wqkh_d.rearrange("p (a h) -> p a h", a=ND))
            nc.scalar.dma_start(wqkl[:], wqkl_d.rearrange("p (a h) -> p a h", a=ND))
            nc.scalar.dma_start(wvh[:], wvh_d.rearrange("p (a h) -> p a h", a=ND))
            nc.scalar.dma_start(identf[:], identf_d[:])
            nc.scalar.dma_start(mask[:], mask_d[:])
            # KHL: rows 0:64 = k_lo, rows 64:128 = k_hi
            KHL = big.tile([P, T], bf16)
            vTb = big.tile([HS, T], bf16)
            vn = big.tile([P, NT, HS], bf16)
            qhh = big.tile([P, 2, 512], bf16)   # rows 0:64=q_hi, 64:128=q_hi
            qlz = big.tile([P, 2, 512], bf16)   # rows 0:64=0,    64:128=q_lo
            nc.vector.memset(qlz[0:HS, :, :], 0.0)

            for ch in range(1, NCH):
                for dt in range(ND):
                    nc.sync.dma_start(
                        xc[ch][:, dt, :], xhl_d[dt * P:(dt + 1) * P, ch, :])

            with tc.tile_pool(name="warm", bufs=1, space="PSUM") as warm:
                dps = warm.tile([P, P], f32, name="dps")
                for _ in range(72):
                    nc.tensor.matmul(dps[:], lhsT=identb[:], rhs=identb[:],
                                     start=True, stop=True)

            with tc.tile_pool(name="spool", bufs=5, space="PSUM") as spool, \
                 tc.tile_pool(name="etp", bufs=2, space="PSUM") as etp, \
                 tc.tile_pool(name="avp", bufs=1, space="PSUM") as avp, \
                 tc.tile_pool(name="epool", bufs=4) as epool, \
                 tc.tile_pool(name="ets", bufs=6) as ets, \
                 tc.tile_pool(name="small", bufs=4) as small, \
                 tc.tile_pool(name="osb", bufs=3) as osb, \
                 tc.tile_pool(name="qtmp", bufs=3) as qtp:

                def emit_proj(role):
                    qchunks = ROLE_QCHUNKS[role]
                    for ch in range(ROLE_NKV[role]):
                        cs = slice(ch * 512, (ch + 1) * 512)
                        is_q = ch in qchunks
                        w_h, w_l = (wqkh, wqkl) if is_q else (wvkh, wvkl)
                        ps = spool.tile([P, 512], f32, tag="S")
                        ngrp = [(w_h, 0), (w_h, 512), (w_l, 0)]
                        n = len(ngrp) * ND
                        i = 0
                        for w_t, xoff in ngrp:
                            for dt in range(ND):
                                nc.tensor.matmul(
                                    ps[:], lhsT=w_t[:, dt, :],
                                    rhs=xc[ch][:, dt, xoff:xoff + 512],
                                    start=(i == 0), stop=(i == n - 1))
                                i += 1
                        # rows 64:128 = k^T always; split k into bf16 pair
                        nc.scalar.copy(KHL[HS:P, cs], ps[HS:P, :])
                        kl = qtp.tile([P, 512], bf16, tag="kl")
                        nc.vector.tensor_sub(kl[HS:P, :], ps[HS:P, :],
                                             KHL[HS:P, cs])
                        nc.gpsimd.dma_start(KHL[0:HS, cs], kl[HS:P, :])
                        if is_q:
                            qc = qchunks.index(ch)
                            nc.scalar.copy(qhh[0:HS, qc, :], ps[0:HS, :])
                            qt = qtp.tile([HS, 512], bf16, tag="qt")
                            nc.vector.tensor_sub(qt[:], ps[0:HS, :],
                                                 qhh[0:HS, qc, :])
                            nc.gpsimd.dma_start(qhh[HS:P, qc, :],
                                                qhh[0:HS, qc, :])
                            nc.gpsimd.dma_start(qlz[HS:P, qc, :], qt[:])
                            # v-only single-group pass for this chunk
                            psv = spool.tile([HS, 512], f32, tag="S")
                            for dt in range(ND):
                                nc.tensor.matmul(
                                    psv[:], lhsT=wvh[:, dt, :],
                                    rhs=xc[ch][:, dt, 0:512],
                                    start=(dt == 0), stop=(dt == ND - 1))
                            nc.vector.tensor_copy(vTb[:, cs], psv[:])
                        else:
                            nc.vector.tensor_copy(vTb[:, cs], ps[0:HS, :])
                        # v^T -> v natural (bf16), 4 transposes in one tile
                        vp = etp.tile([P, 512], bf16, tag="ep")
                        for u in range(4):
                            tt = ch * 4 + u
                            nc.tensor.transpose(
                                vp[:, u * HS:u * HS + HS],
                                vTb[:, tt * P:(tt + 1) * P],
                                identb[0:HS, 0:HS])
                        nc.scalar.copy(
                            vn[:, ch * 4:(ch + 1) * 4, :],
                            vp[:, 0:4 * HS].rearrange("p (a h) -> p a h", a=4))

                def emit_role(role):
                    emit_proj(role)

                    def chunk_scores(st, kc):
                        L = st["L"]
                        w = min(512, L - kc * 512)
                        sp = spool.tile([P, w], f32, tag="S")
                        rhs = KHL[:, kc * 512:kc * 512 + w]
                        nc.tensor.matmul(sp[:], lhsT=st["qh"], rhs=rhs,
                                         start=True, stop=False)
                        nc.tensor.matmul(sp[:], lhsT=st["ql"], rhs=rhs,
                                         start=False, stop=True)
                        if kc == st["nch"] - 1:
                            nc.vector.tensor_add(
                                sp[:, w - P:w], sp[:, w - P:w], mask[:])
                        nc.vector.reduce_max(
                            st["mc"][:, kc:kc + 1], sp[:], axis=AX)
                        st["sps"].append((sp, w))

                    def begin_block(slot, j):
                        L = 128 * (j + 1)
                        qc, off = _block_qloc(role, j)
                        st = {"slot": slot, "j": j, "L": L,
                              "qh": qhh[:, qc, off:off + 128],
                              "ql": qlz[:, qc, off:off + 128],
                              "nch": (L + 511) // 512, "sps": []}
                        st["mc"] = small.tile([P, st["nch"]], f32, tag="mc",
                                              name="mc")
                        chunk_scores(st, 0)
                        return st

                    def emit_tail(st):
                        slot, L, nch = st["slot"], st["L"], st["nch"]
                        nm8 = small.tile([P, 1], f32, tag="nm8")
                        if nch == 1:
                            nc.vector.tensor_scalar_mul(
                                nm8[:], st["mc"][:, 0:1], -SCALE)
                        else:
                            m = small.tile([P, 1], f32, tag="m")
                            nc.vector.reduce_max(m[:], st["mc"][:], axis=AX)
                            nc.vector.tensor_scalar_mul(nm8[:], m[:], -SCALE)

                        E = epool.tile([P, L], bf16, tag="E")
                        zc = small.tile([P, nch], f32, tag="zc", name="zc")
                        for kc, (sp, w) in enumerate(st["sps"]):
                            nc.scalar.activation(
                                E[:, kc * 512:kc * 512 + w], sp[:], EXP,
                                bias=nm8[:], scale=SCALE,
                                accum_out=zc[:, kc:kc + 1])

                        rz = small.tile([P, 1], f32, tag="rz")
                        if nch == 1:
                            nc.vector.reciprocal(rz[:], zc[:, 0:1])
                        else:
                            zs = small.tile([P, 1], f32, tag="zs")
                            nc.vector.reduce_sum(zs[:], zc[:], axis=AX)
                            nc.vector.reciprocal(rz[:], zs[:])

                        # E^T via PE transposes, 8 per PSUM tile, one copy
                        av = avp.tile([HS, P], f32, tag="av")
                        nkt = L // P
                        kt = 0
                        gi = 0
                        while kt < nkt:
                            gn = min(8, nkt - kt)
                            ep = etp.tile([P, 1024], bf16, tag="ep")
                            for u in range(gn):
                                nc.tensor.transpose(
                                    ep[:, u * P:(u + 1) * P],
                                    E[:, (kt + u) * P:(kt + u + 1) * P],
                                    identb[:])
                            es = ets.tile([P, 1024], bf16, tag="ets")
                            if gi % 3 == 0:
                                nc.scalar.copy(es[:, 0:gn * P],
                                               ep[:, 0:gn * P])
                            else:
                                nc.vector.tensor_copy(es[:, 0:gn * P],
                                                      ep[:, 0:gn * P])
                            for u in range(gn):
                                nc.tensor.matmul(
                                    av[:], lhsT=vn[:, kt + u, :],
                                    rhs=es[:, u * P:(u + 1) * P],
                                    start=(kt + u == 0),
                                    stop=(kt + u == nkt - 1),
                                    skip_group_check=True)
                            kt += gn
                            gi += 1

                        avs = osb.tile([HS, P], f32, tag="avs")
                        nc.vector.tensor_copy(avs[:], av[:])
                        op = etp.tile([P, HS], f32, tag="ep")
                        nc.tensor.transpose(op[:], avs[:], identf[:])
                        ob = osb.tile([P, HS], f32, tag="ob")
                        nc.vector.tensor_scalar_mul(ob[:], op[:], rz[:])
                        nc.sync.dma_start(
                            out_d[slot * P:(slot + 1) * P, :], ob[:])

                    prev = None
                    for slot, j in enumerate(ROLE_BLOCKS[role]):
                        st = begin_block(slot, j)
                        # hoist up to 5 - nch(prev) score chunks of this
                        # block ahead of prev's softmax tail (PSUM budget)
                        hoist = 1
                        if prev is not None and prev["nch"] <= 3                                 and st["nch"] >= 2:
                            chunk_scores(st, 1)
                            hoist = 2
                        if prev is not None:
                            emit_tail(prev)
                        for kc in range(hoist, st["nch"]):
                            chunk_scores(st, kc)
                        prev = st
                    emit_tail(prev)

                pid = nc.partition_id()
                with tc.If(pid < 4) as cmp:
                    emit_role(0)
                with cmp.Else():
                    emit_role(1)

    nc.compile()
    return nc


def _get_program():
    global _COMPILED
    if _COMPILED is None:
        _COMPILED = _build()
    return _COMPILED


def _install_ntff_hook():
    import sys, types
    if "antenv.axon_hooks" in sys.modules:
        return
    try:
        from trn_agent_boot.trn_boot import _ntff_profile_via_ctypes
        hook = _ntff_profile_via_ctypes("/opt/axon/libaxon_pjrt.so")
        mod = types.ModuleType("antenv.axon_hooks")
        mod.get_axon_ntff_profile_hook = lambda: hook
        mod.set_axon_ntff_profile_hook = lambda h: None
        import antenv
        sys.modules["antenv.axon_hooks"] = mod
        antenv.axon_hooks = mod
    except Exception:
        pass


def _split_pair(a):
    import ml_dtypes
    hi = a.astype(ml_dtypes.bfloat16)
    lo = (a - hi.astype(np.float32)).astype(ml_dtypes.bfloat16)
    return hi, lo


def _host_prep(inputs):
    import ml_dtypes
    x = np.asarray(inputs["x"], dtype=np.float32)
    wq = np.asarray(inputs["Wq"], dtype=np.float32)
    wk = np.asarray(inputs["Wk"], dtype=np.float32)
    wv = np.asarray(inputs["Wv"], dtype=np.float32)

    xt = np.transpose(x, (0, 2, 1))                # [B, D, T]
    xtc = xt.reshape(B, D, NCH, 512)               # chunked over T
    hi, lo = _split_pair(np.ascontiguousarray(xtc))
    xhl = np.concatenate([hi, lo], axis=3)         # [B, D, NCH, 1024]

    def _wprep(wt):
        # [D, M] -> [P, ND*M]: row p holds [dt, m] for d = dt*P + p
        m = wt.shape[1]
        return np.ascontiguousarray(
            wt.reshape(ND, P, m).transpose(1, 0, 2).reshape(P, ND * m))

    wvkh, wvkl = _split_pair(_wprep(np.concatenate([wv, wk], axis=0).T))
    wqkh, wqkl = _split_pair(_wprep(np.concatenate([wq, wk], axis=0).T))
    wvhh, _ = _split_pair(_wprep(wv.T))

    identb = np.eye(P, dtype=ml_dtypes.bfloat16)
    identf = np.eye(HS, dtype=np.float32)
    r = np.arange(P)
    mask = np.where(r[None, :] <= r[:, None], 0.0, NEG).astype(np.float32)

    shared = {"wvkh": wvkh, "wvkl": wvkl, "wqkh": wqkh, "wqkl": wqkl,
              "wvh": wvhh, "identb": identb, "identf": identf, "mask": mask}
    in_maps = []
    for c in range(N_CORES):
        m = dict(shared)
        m["xhl"] = np.ascontiguousarray(xhl[c % B])
        in_maps.append(m)
    return in_maps


def _run(inputs, trace=False):
    from concourse.bass_utils import run_bass_kernel_spmd

    if trace:
        _install_ntff_hook()
    nc = _get_program()
    in_maps = _host_prep(inputs)
    res = run_bass_kernel_spmd(nc, in_maps, list(range(N_CORES)), trace=trace)

    out = np.empty((B, T, HS), dtype=np.float32)
    for c in range(N_CORES):
        b, role = c % B, c // B
        oc = res.results[c]["out"]
        for slot, j in enumerate(ROLE_BLOCKS[role]):
            out[b, 128 * j:128 * (j + 1)] = oc[128 * slot:128 * (slot + 1)]
    return out, res


def kernel(**inputs):
    out, _ = _run(inputs, trace=False)
    return out


# revision 36
# speedup vs baseline: 1.0475x; 1.0019x over previous
"""Single-head causal attention (B=4, T=2048, D=1024, HS=64) on 8 TRN2 cores.

Sharding: 2 cores per batch element. Query blocks (128 rows, 16/batch) are
fold-split for perfect causal balance:
  role 0 (cores 0-3): blocks {0,1,2,3,12,13,14,15} of batch (core_id % 4)
  role 1 (cores 4-7): blocks {4..11}              of batch (core_id % 4)
The SPMD program branches on partition_id for the role-specific part.

Precision scheme (bf16 hi/lo pairs, ~17-18 effective mantissa bits):
  host: x.T split into bf16 hi/lo, interleaved per 512-column chunk
  (xhl[d, ch, 0:512]=hi, [512:1024]=lo) so DMA rows stay 2KB contiguous.
  W pre-transposed bf16 pairs.  Projections per 512-col chunk (role-dep):
    q-chunks   : lhsT=[Wq|Wk] 3 groups -> PSUM rows 0:64 q^T, 64:128 k^T,
                 plus a 1-group v-only pass (lhsT=[Wv_hi], v to bf16 acc).
    other chunk: lhsT=[Wv|Wk] 3 groups -> v^T, k^T.
  (3 groups = xh*wh + xl*wh + xh*wl accumulated in fp32 PSUM.)
  role 1 needs keys only to t<1536: its chunk-3 k/v pass is skipped.
  scores: k,q re-split into bf16 pairs on device;
    S = [qh;qh]^T.[kl;kh] + [0;ql]^T.[kl;kh]  (2 matmuls per 512-chunk)
  softmax: chunked row-max (DVE) + exp on ACT (scale=8, bias=-8*max),
  E bf16; E^T via PE transposes batched 4-per-PSUM-tile; out^T = v^T @ E^T
  in PSUM; 1/Z applied after a small back-transpose. Output fp32.
"""

import numpy as np

N_CORES = 8
B, T, D, HS = 4, 2048, 1024, 64
P = 128
NT = T // P        # 16
ND = D // P        # 8
NCH = 4            # 512-wide t chunks
SCALE = 8.0        # sqrt(HS)
NEG = -1.0e30

ROLE_BLOCKS = [
    [3, 12, 1, 13, 2, 14, 15, 0],
    [4, 8, 5, 9, 6, 10, 11, 7],
]
ROLE_QCHUNKS = [[0, 3], [1, 2]]  # 512-chunk indices holding each role's q rows
ROLE_NKV = [4, 3]                # k/v chunks needed per role


def _block_qloc(role, j):
    if role == 0:
        return (0, 128 * j) if j < 4 else (1, 128 * (j - 12))
    return (0, 128 * (j - 4)) if j < 8 else (1, 128 * (j - 8))


_COMPILED = None


def _build():
    import concourse.bass as bass
    import concourse.tile as tile
    from concourse import bacc, mybir

    f32 = mybir.dt.float32
    bf16 = mybir.dt.bfloat16
    EXP = mybir.ActivationFunctionType.Exp
    AX = mybir.AxisListType.X

    nc = bacc.Bacc("TRN2", target_bir_lowering=False, debug=False,
                   num_devices=N_CORES)

    xhl_d = nc.dram_tensor("xhl", [D, NCH, 1024], bf16,
                           kind="ExternalInput").ap()
    wvkh_d = nc.dram_tensor("wvkh", [P, ND * P], bf16, kind="ExternalInput").ap()
    wvkl_d = nc.dram_tensor("wvkl", [P, ND * P], bf16, kind="ExternalInput").ap()
    wqkh_d = nc.dram_tensor("wqkh", [P, ND * P], bf16, kind="ExternalInput").ap()
    wqkl_d = nc.dram_tensor("wqkl", [P, ND * P], bf16, kind="ExternalInput").ap()
    wvh_d = nc.dram_tensor("wvh", [P, ND * HS], bf16, kind="ExternalInput").ap()
    identb_d = nc.dram_tensor("identb", [P, P], bf16, kind="ExternalInput").ap()
    identf_d = nc.dram_tensor("identf", [HS, HS], f32, kind="ExternalInput").ap()
    mask_d = nc.dram_tensor("mask", [P, P], f32, kind="ExternalInput").ap()
    out_d = nc.dram_tensor("out", [1024, HS], f32, kind="ExternalOutput").ap()

    with tile.TileContext(nc) as tc:
        with tc.tile_pool(name="consts", bufs=1) as consts, \
             tc.tile_pool(name="big", bufs=1) as big:
            identb = consts.tile([P, P], bf16)
            identf = consts.tile([HS, HS], f32)
            mask = consts.tile([P, P], f32)
            wvkh = consts.tile([P, ND, P], bf16)
            wvkl = consts.tile([P, ND, P], bf16)
            wqkh = consts.tile([P, ND, P], bf16)
            wqkl = consts.tile([P, ND, P], bf16)
            wvh = consts.tile([P, ND, HS], bf16)

            # per-chunk x^T tiles: [:, dt, 0:512] = hi, [512:1024] = lo
            xc = [big.tile([P, ND, 1024], bf16, name=f"xc{ch}", tag=f"xc{ch}")
                  for ch in range(NCH)]
            # identity first (gates the PE warm-up chain), then x chunk 0
            nc.sync.dma_start(identb[:], identb_d[:])
            for dt in range(ND):
                nc.sync.dma_start(
                    xc[0][:, dt, :], xhl_d[dt * P:(dt + 1) * P, 0, :])
            nc.scalar.dma_start(wvkh[:], wvkh_d.rearrange("p (a h) -> p a h", a=ND))
            nc.scalar.dma_start(wvkl[:], wvkl_d.rearrange("p (a h) -> p a h", a=ND))
            nc.scalar.dma_start(wqkh[:], wqkh_d.rearrange("p (a h) -> p a h", a=ND))
            nc.scalar.dma_start(wqkl[:], wqkl_d.rearrange("p (a h) -> p a h", a=ND))
            nc.scalar.dma_start(wvh[:], wvh_d.rearrange("p (a h) -> p a h", a=ND))
            nc.scalar.dma_start(identf[:], identf_d[:])
            nc.scalar.dma_start(mask[:], mask_d[:])
            # KHL: rows 0:64 = k_lo, rows 64:128 = k_hi
            KHL = big.tile([P, T], bf16)
            vTb = big.tile([HS, T], bf16)
            vn = big.tile([P, NT, HS], bf16)
            qhh = big.tile([P, 2, 512], bf16)   # rows 0:64=q_hi, 64:128=q_hi
            qlz = big.tile([P, 2, 512], bf16)   # rows 0:64=0,    64:128=q_lo
            nc.vector.memset(qlz[0:HS, :, :], 0.0)

            for ch in range(1, NCH):
                for dt in range(ND):
                    nc.sync.dma_start(
                        xc[ch][:, dt, :], xhl_d[dt * P:(dt + 1) * P, ch, :])

            with tc.tile_pool(name="warm", bufs=1, space="PSUM") as warm:
                dps = warm.tile([P, P], f32, name="dps")
                for _ in range(72):
                    nc.tensor.matmul(dps[:], lhsT=identb[:], rhs=identb[:],
                                     start=True, stop=True)

            with tc.tile_pool(name="spool", bufs=5, space="PSUM") as spool, \
                 tc.tile_pool(name="etp", bufs=2, space="PSUM") as etp, \
                 tc.tile_pool(name="avp", bufs=1, space="PSUM") as avp, \
                 tc.tile_pool(name="epool", bufs=4) as epool, \
                 tc.tile_pool(name="ets", bufs=6) as ets, \
                 tc.tile_pool(name="small", bufs=4) as small, \
                 tc.tile_pool(name="osb", bufs=3) as osb, \
                 tc.tile_pool(name="qtmp", bufs=3) as qtp:

                def emit_proj(role):
                    qchunks = ROLE_QCHUNKS[role]
                    for ch in range(ROLE_NKV[role]):
                        cs = slice(ch * 512, (ch + 1) * 512)
                        is_q = ch in qchunks
                        w_h, w_l = (wqkh, wqkl) if is_q else (wvkh, wvkl)
                        ps = spool.tile([P, 512], f32, tag="S")
                        ngrp = [(w_h, 0), (w_h, 512), (w_l, 0)]
                        n = len(ngrp) * ND
                        i = 0
                        for w_t, xoff in ngrp:
                            for dt in range(ND):
                                nc.tensor.matmul(
                                    ps[:], lhsT=w_t[:, dt, :],
                                    rhs=xc[ch][:, dt, xoff:xoff + 512],
                                    start=(i == 0), stop=(i == n - 1))
                                i += 1
                        # rows 64:128 = k^T always; split k into bf16 pair
                        nc.scalar.copy(KHL[HS:P, cs], ps[HS:P, :])
                        kl = qtp.tile([P, 512], bf16, tag="kl")
                        nc.vector.tensor_sub(kl[HS:P, :], ps[HS:P, :],
                                             KHL[HS:P, cs])
                        nc.gpsimd.dma_start(KHL[0:HS, cs], kl[HS:P, :])
                        if is_q:
                            qc = qchunks.index(ch)
                            nc.scalar.copy(qhh[0:HS, qc, :], ps[0:HS, :])
                            qt = qtp.tile([HS, 512], bf16, tag="qt")
                            nc.vector.tensor_sub(qt[:], ps[0:HS, :],
                                                 qhh[0:HS, qc, :])
                            nc.gpsimd.dma_start(qhh[HS:P, qc, :],
                                                qhh[0:HS, qc, :])
                            nc.gpsimd.dma_start(qlz[HS:P, qc, :], qt[:])
                            # v-only single-group pass for this chunk
                            psv = spool.tile([HS, 512], f32, tag="S")
                            for dt in range(ND):
                                nc.tensor.matmul(
                                    psv[:], lhsT=wvh[:, dt, :],
                                    rhs=xc[ch][:, dt, 0:512],
                                    start=(dt == 0), stop=(dt == ND - 1))
                            nc.vector.tensor_copy(vTb[:, cs], psv[:])
                        else:
                            nc.vector.tensor_copy(vTb[:, cs], ps[0:HS, :])
                        # v^T -> v natural (bf16), 4 transposes in one tile
                        vp = etp.tile([P, 512], bf16, tag="ep")
                        for u in range(4):
                            tt = ch * 4 + u
                            nc.tensor.transpose(
                                vp[:, u * HS:u * HS + HS],
                                vTb[:, tt * P:(tt + 1) * P],
                                identb[0:HS, 0:HS])
                        nc.scalar.copy(
                            vn[:, ch * 4:(ch + 1) * 4, :],
                            vp[:, 0:4 * HS].rearrange("p (a h) -> p a h", a=4))

                def emit_role(role):
                    emit_proj(role)

                    def chunk_scores(st, kc):
                        L = st["L"]
                        w = min(512, L - kc * 512)
                        sp = spool.tile([P, w], f32, tag="S")
                        rhs = KHL[:, kc * 512:kc * 512 + w]
                        nc.tensor.matmul(sp[:], lhsT=st["qh"], rhs=rhs,
                                         start=True, stop=False)
                        nc.tensor.matmul(sp[:], lhsT=st["ql"], rhs=rhs,
                                         start=False, stop=True)
                        if kc == st["nch"] - 1:
                            nc.vector.tensor_add(
                                sp[:, w - P:w], sp[:, w - P:w], mask[:])
                        nc.vector.reduce_max(
                            st["mc"][:, kc:kc + 1], sp[:], axis=AX)
                        st["sps"].append((sp, w))

                    def begin_block(slot, j):
                        L = 128 * (j + 1)
                        qc, off = _block_qloc(role, j)
                        st = {"slot": slot, "j": j, "L": L,
                              "qh": qhh[:, qc, off:off + 128],
                              "ql": qlz[:, qc, off:off + 128],
                              "nch": (L + 511) // 512, "sps": []}
                        st["mc"] = small.tile([P, st["nch"]], f32, tag="mc",
                                              name="mc")
                        chunk_scores(st, 0)
                        return st

                    def emit_tail(st):
                        slot, L, nch = st["slot"], st["L"], st["nch"]
                        nm8 = small.tile([P, 1], f32, tag="nm8")
                        if nch == 1:
                            nc.vector.tensor_scalar_mul(
                                nm8[:], st["mc"][:, 0:1], -SCALE)
                        else:
                            m = small.tile([P, 1], f32, tag="m")
                            nc.vector.reduce_max(m[:], st["mc"][:], axis=AX)
                            nc.vector.tensor_scalar_mul(nm8[:], m[:], -SCALE)

                        E = epool.tile([P, L], bf16, tag="E")
                        zc = small.tile([P, nch], f32, tag="zc", name="zc")
                        for kc, (sp, w) in enumerate(st["sps"]):
                            nc.scalar.activation(
                                E[:, kc * 512:kc * 512 + w], sp[:], EXP,
                                bias=nm8[:], scale=SCALE,
                                accum_out=zc[:, kc:kc + 1])

                        rz = small.tile([P, 1], f32, tag="rz")
                        if nch == 1:
                            nc.vector.reciprocal(rz[:], zc[:, 0:1])
                        else:
                            zs = small.tile([P, 1], f32, tag="zs")
                            nc.vector.reduce_sum(zs[:], zc[:], axis=AX)
                            nc.vector.reciprocal(rz[:], zs[:])

                        # E^T via PE transposes, 8 per PSUM tile, one copy
                        av = avp.tile([HS, P], f32, tag="av")
                        nkt = L // P
                        kt = 0
                        gi = 0
                        while kt < nkt:
                            gn = min(8, nkt - kt)
                            ep = etp.tile([P, 1024], bf16, tag="ep")
                            for u in range(gn):
                                nc.tensor.transpose(
                                    ep[:, u * P:(u + 1) * P],
                                    E[:, (kt + u) * P:(kt + u + 1) * P],
                                    identb[:])
                            es = ets.tile([P, 1024], bf16, tag="ets")
                            if gi % 3 == 0:
                                nc.scalar.copy(es[:, 0:gn * P],
                                               ep[:, 0:gn * P])
                            else:
                                nc.vector.tensor_copy(es[:, 0:gn * P],
                                                      ep[:, 0:gn * P])
                            for u in range(gn):
                                nc.tensor.matmul(
                                    av[:], lhsT=vn[:, kt + u, :],
                                    rhs=es[:, u * P:(u + 1) * P],
                                    start=(kt + u == 0),
                                    stop=(kt + u == nkt - 1),
                                    skip_group_check=True)
                            kt += gn
                            gi += 1

                        avs = osb.tile([HS, P], f32, tag="avs")
                        nc.vector.tensor_copy(avs[:], av[:])
                        op = etp.tile([P, HS], f32, tag="ep")
                        nc.tensor.transpose(op[:], avs[:], identf[:])
                        ob = osb.tile([P, HS], f32, tag="ob")
                        nc.vector.tensor_scalar_mul(ob[:], op[:], rz[:])
                        nc.sync.dma_start(
                            out_d[slot * P:(slot + 1) * P, :], ob[:])

                    prev = None
                    for slot, j in enumerate(ROLE_BLOCKS[role]):
                        st = begin_block(slot, j)
                        if prev is not None:
                            emit_tail(prev)
                        for kc in range(1, st["nch"]):
                            chunk_scores(st, kc)
                        prev = st
                    emit_tail(prev)

                pid = nc.partition_id()
                with tc.If(pid < 4) as cmp:
                    emit_role(0)
                with cmp.Else():
                    emit_role(1)

    nc.compile()
    return nc


def _get_program():
    global _COMPILED
    if _COMPILED is None:
        _COMPILED = _build()
    return _COMPILED


def _install_ntff_hook():
    import sys, types
    if "antenv.axon_hooks" in sys.modules:
        return
    try:
        from trn_agent_boot.trn_boot import _ntff_profile_via_ctypes
        hook = _ntff_profile_via_ctypes("/opt/axon/libaxon_pjrt.so")
        mod = types.ModuleType("antenv.axon_hooks")
        mod.get_axon_ntff_profile_hook = lambda: hook
        mod.set_axon_ntff_profile_hook = lambda h: None
        import antenv
        sys.modules["antenv.axon_hooks"] = mod
        antenv.axon_hooks = mod
    except Exception:
        pass


def _split_pair(a):
    import ml_dtypes
    hi = a.astype(ml_dtypes.bfloat16)
    lo = (a - hi.astype(np.float32)).astype(ml_dtypes.bfloat16)
    return hi, lo


def _host_prep(inputs):
    import ml_dtypes
    x = np.asarray(inputs["x"], dtype=np.float32)
    wq = np.asarray(inputs["Wq"], dtype=np.float32)
    wk = np.asarray(inputs["Wk"], dtype=np.float32)
    wv = np.asarray(inputs["Wv"], dtype=np.float32)

    xt = np.transpose(x, (0, 2, 1))                # [B, D, T]
    xtc = xt.reshape(B, D, NCH, 512)               # chunked over T
    hi, lo = _split_pair(np.ascontiguousarray(xtc))
    xhl = np.concatenate([hi, lo], axis=3)         # [B, D, NCH, 1024]

    def _wprep(wt):
        # [D, M] -> [P, ND*M]: row p holds [dt, m] for d = dt*P + p
        m = wt.shape[1]
        return np.ascontiguousarray(
            wt.reshape(ND, P, m).transpose(1, 0, 2).reshape(P, ND * m))

    wvkh, wvkl = _split_pair(_wprep(np.concatenate([wv, wk], axis=0).T))
    wqkh, wqkl = _split_pair(_wprep(np.concatenate([wq, wk], axis=0).T))
    wvhh, _ = _split_pair(_wprep(wv.T))

    identb = np.eye(P, dtype=ml_dtypes.bfloat16)
    identf = np.eye(HS, dtype=np.float32)
    r = np.arange(P)
    mask = np.where(r[None, :] <= r[:, None], 0.0, NEG).astype(np.float32)

    shared = {"wvkh": wvkh, "wvkl": wvkl, "wqkh": wqkh, "wqkl": wqkl,
              "wvh": wvhh, "identb": identb, "identf": identf, "mask": mask}
    in_maps = []
    for c in range(N_CORES):
        m = dict(shared)
        m["xhl"] = np.ascontiguousarray(xhl[c % B])
        in_maps.append(m)
    return in_maps


def _run(inputs, trace=False):
    from concourse.bass_utils import run_bass_kernel_spmd

    if trace:
        _install_ntff_hook()
    nc = _get_program()
    in_maps = _host_prep(inputs)
    res = run_bass_kernel_spmd(nc, in_maps, list(range(N_CORES)), trace=trace)

    out = np.empty((B, T, HS), dtype=np.float32)
    for c in range(N_CORES):
        b, role = c % B, c // B
        oc = res.results[c]["out"]
        for slot, j in enumerate(ROLE_BLOCKS[role]):
            out[b, 128 * j:128 * (j + 1)] = oc[128 * slot:128 * (slot + 1)]
    return out, res


def kernel(**inputs):
    out, _ = _run(inputs, trace=False)
    return out
